# revision 5
# baseline (speedup 1.0000x reference)
"""AudioGRU Trainium2 Bass kernel.

Single-layer GRU (PyTorch gate order r,z,n) over T=2000 steps followed by a
mean over time. Data-parallel over the batch axis across 8 NeuronCores
(B=256 -> 32 per core); weights replicated; the time recurrence is local.

Layout: everything lives transposed on-chip, [H=128 partitions, batch free].
Per step the three gate pre-activations gh = W_hh @ h accumulate into PSUM
banks on top of the input projections gx = W_ih @ x_t, which a "sweep" of
small matmuls computes 16 steps ahead of the recurrence using PE idle time.
Gate math: r,z sigmoids on ACT (fp32 per-partition biases), the n-gate uses a
fused (gh_n + b_hh_n) * r on the vector engine, tanh on ACT. The recurrence
state h is bf16 (validated: absmax rel err ~1e-3 vs fp32 reference).
"""

import os
import sys
import numpy as np
import ml_dtypes
from contextlib import ExitStack

for _p in ("/opt/trn_rl_repo", "/root/.axon_site/_ro/trn_rl_repo"):
    if os.path.isdir(_p) and _p not in sys.path:
        sys.path.insert(0, _p)

B, T, I, H = 256, 2000, 23, 128
NCORES = 8
BL = B // NCORES          # 32 batch per core
BLK = 16                  # gx psum block: 16 steps * 32 batch = 512 f32 = one bank
CHUNK = 256               # x DMA chunk, in timesteps (multiple of BLK)
bf16 = ml_dtypes.bfloat16

_PROG_CACHE = {}


def _emit(ctx, tc, nc, xT, wih, whh, brz, bn, yT, T_):
    import concourse.bass as bass  # noqa: F401
    from concourse import mybir

    f32, b16 = mybir.dt.float32, mybir.dt.bfloat16
    AF = mybir.ActivationFunctionType
    OP = mybir.AluOpType
    NBLK = T_ // BLK
    nchunk = (T_ + CHUNK - 1) // CHUNK

    const = ctx.enter_context(tc.tile_pool(name="const", bufs=1))
    xpool = ctx.enter_context(tc.tile_pool(name="xp", bufs=3))
    gxp_r = ctx.enter_context(tc.tile_pool(name="gxr", bufs=2, space="PSUM"))
    gxp_z = ctx.enter_context(tc.tile_pool(name="gxz", bufs=2, space="PSUM"))
    gxp_n = ctx.enter_context(tc.tile_pool(name="gxn", bufs=2, space="PSUM"))
    ghp = ctx.enter_context(tc.tile_pool(name="ghp", bufs=1, space="PSUM"))
    work = ctx.enter_context(tc.tile_pool(name="wk", bufs=3))

    wih_sb = const.tile([I, 3 * H], b16, name="wih_sb")
    nc.sync.dma_start(wih_sb[:], wih)
    whh_sb = const.tile([H, 3 * H], b16, name="whh_sb")
    nc.sync.dma_start(whh_sb[:], whh)
    brz_sb = const.tile([H, 2], f32, name="brz_sb")
    nc.sync.dma_start(brz_sb[:], brz)
    bn_sb = const.tile([H, 2], f32, name="bn_sb")
    nc.sync.dma_start(bn_sb[:], bn)

    h = const.tile([H, BL], b16, name="h_state")
    nc.vector.memset(h[:], 0.0)
    hs = const.tile([H, BL], f32, name="h_sum")
    nc.vector.memset(hs[:], 0.0)

    xs = []

    def load_chunk(c):
        steps = min(CHUNK, T_ - c * CHUNK)
        xc = xpool.tile([I, steps * BL], b16, name="xc", tag="xc")
        nc.sync.dma_start(xc[:], xT[:, c * CHUNK : c * CHUNK + steps, :])
        return xc

    xs.append(load_chunk(0))
    if nchunk > 1:
        xs.append(load_chunk(1))

    # gh_n scratch bank: two rotating [H, BL] slots
    ghn = ghp.tile([H, 512], f32, name="ghn_bank")

    def alloc_block():
        gr = gxp_r.tile([H, BLK * BL], f32, name="gr", tag="gr")
        gz = gxp_z.tile([H, BLK * BL], f32, name="gz", tag="gz")
        gn = gxp_n.tile([H, BLK * BL], f32, name="gn", tag="gn")
        return (gr, gz, gn)

    def sweep_block(blk, b):
        # Input projections for block b, one full-bank matmul per gate.
        # start=True opens the bank's accumulation group; the r/z banks stay
        # open for the per-step gh accumulation (closed at the block's last
        # step); gx_n receives no accumulation, close immediately.
        t0 = b * BLK
        c, o = divmod(t0, CHUNK)
        rhs = xs[c][:, o * BL : (o + BLK) * BL]
        for g in range(3):
            nc.tensor.matmul(
                blk[g][:, : BLK * BL],
                wih_sb[:, g * H : (g + 1) * H],
                rhs,
                start=True,
                stop=(g == 2),
                skip_group_check=True,
            )

    blk_cur = alloc_block()
    sweep_block(blk_cur, 0)

    blk_next = None
    for t in range(T_):
        b_, j = divmod(t, BLK)
        if j == 0:
            if t % CHUNK == 0 and t // CHUNK + 2 < nchunk:
                xs.append(load_chunk(t // CHUNK + 2))
            blk_next = alloc_block() if b_ + 1 < NBLK else None
            if blk_next is not None:
                sweep_block(blk_next, b_ + 1)

        gr, gz, gn = blk_cur
        sl = slice(j * BL, (j + 1) * BL)
        slot = (t % 2) * BL
        last = j == BLK - 1
        # Recurrence matmuls: accumulate gh_r/gh_z onto gx banks; gh_n alone.
        nc.tensor.matmul(gr[:, sl], whh_sb[:, 0:H], h[:], start=False, stop=last, skip_group_check=True)
        nc.tensor.matmul(gz[:, sl], whh_sb[:, H : 2 * H], h[:], start=False, stop=last, skip_group_check=True)
        nc.tensor.matmul(
            ghn[:, slot : slot + BL], whh_sb[:, 2 * H : 3 * H], h[:], start=True, stop=True,
            skip_group_check=True,
        )

        r = work.tile([H, BL], f32, name="r", tag="r")
        nc.scalar.activation(r[:], gr[:, sl], AF.Sigmoid, bias=brz_sb[:, 0:1])
        z = work.tile([H, BL], f32, name="z", tag="z")
        nc.scalar.activation(z[:], gz[:, sl], AF.Sigmoid, bias=brz_sb[:, 1:2])

        # t1 = (gh_n + b_hh_n) * r ; t2 = t1 + gx_n ; n = tanh(t2 + b_ih_n)
        t1 = work.tile([H, BL], f32, name="t1", tag="t1")
        nc.vector.scalar_tensor_tensor(
            t1[:], ghn[:, slot : slot + BL], bn_sb[:, 1:2], r[:], OP.add, OP.mult
        )
        t2 = work.tile([H, BL], f32, name="t2", tag="t2")
        nc.vector.tensor_tensor(t2[:], t1[:], gn[:, sl], OP.add)
        nn = work.tile([H, BL], f32, name="nn", tag="nn")
        nc.scalar.activation(nn[:], t2[:], AF.Tanh, bias=bn_sb[:, 0:1])

        # Off-critical z-path on gpsimd: c = 1-z, dd = z*h_prev, and the sum.
        cc = work.tile([H, BL], f32, name="cc", tag="cc")
        nc.gpsimd.tensor_scalar(cc[:], z[:], -1.0, 1.0, OP.mult, OP.add)
        dd = work.tile([H, BL], f32, name="dd", tag="dd")
        nc.gpsimd.tensor_tensor(dd[:], z[:], h[:], OP.mult)

        ncv = work.tile([H, BL], f32, name="ncv", tag="ncv")
        nc.vector.tensor_tensor(ncv[:], nn[:], cc[:], OP.mult)
        nc.vector.tensor_tensor(h[:], ncv[:], dd[:], OP.add)  # h <- (1-z)n + z h
        nc.gpsimd.tensor_tensor(hs[:], hs[:], h[:], OP.add)

        if j == BLK - 1:
            blk_cur = blk_next

    out_sb = const.tile([H, BL], f32, name="out_sb")
    nc.scalar.mul(out_sb[:], hs[:], 1.0 / T_)
    nc.sync.dma_start(yT, out_sb[:])


def build_program(T_=T):
    if T_ in _PROG_CACHE:
        return _PROG_CACHE[T_]
    import concourse.tile as tile
    from concourse import bacc, mybir

    f32, b16 = mybir.dt.float32, mybir.dt.bfloat16
    nc = bacc.Bacc(
        "TRN2", target_bir_lowering=False, debug=False, num_devices=NCORES
    )
    xT = nc.dram_tensor("xT", [I, T_, BL], b16, kind="ExternalInput").ap()
    wih = nc.dram_tensor("wih", [I, 3 * H], b16, kind="ExternalInput").ap()
    whh = nc.dram_tensor("whh", [H, 3 * H], b16, kind="ExternalInput").ap()
    brz = nc.dram_tensor("brz", [H, 2], f32, kind="ExternalInput").ap()
    bn = nc.dram_tensor("bn", [H, 2], f32, kind="ExternalInput").ap()
    yT = nc.dram_tensor("yT", [H, BL], f32, kind="ExternalOutput").ap()

    with tile.TileContext(nc) as tc:
        with ExitStack() as ctx:
            _emit(ctx, tc, nc, xT, wih, whh, brz, bn, yT, T_)
    nc.compile()
    _PROG_CACHE[T_] = nc
    return nc


def make_in_maps(x, W_ih, W_hh, b_ih, b_hh, T_=T):
    x = np.asarray(x, dtype=np.float32)
    W_ih = np.asarray(W_ih, dtype=np.float32)
    W_hh = np.asarray(W_hh, dtype=np.float32)
    b_ih = np.asarray(b_ih, dtype=np.float32)
    b_hh = np.asarray(b_hh, dtype=np.float32)

    wihT = np.ascontiguousarray(W_ih.T).astype(bf16)   # [I, 3H]
    whhT = np.ascontiguousarray(W_hh.T).astype(bf16)   # [H, 3H]
    brz = np.stack(
        [b_ih[0:H] + b_hh[0:H], b_ih[H : 2 * H] + b_hh[H : 2 * H]], axis=1
    ).astype(np.float32)                               # [H, 2]
    bn = np.stack([b_ih[2 * H :], b_hh[2 * H :]], axis=1).astype(np.float32)

    in_maps = []
    for c in range(NCORES):
        xc = x[c * BL : (c + 1) * BL, :T_, :]          # [BL, T, I]
        xTc = np.ascontiguousarray(xc.transpose(2, 1, 0)).astype(bf16)
        in_maps.append({"xT": xTc, "wih": wihT, "whh": whhT, "brz": brz, "bn": bn})
    return in_maps


def run(x, W_ih, W_hh, b_ih, b_hh, T_=T, trace=False, **kw):
    from concourse import bass_utils

    nc = build_program(T_)
    in_maps = make_in_maps(x, W_ih, W_hh, b_ih, b_hh, T_)
    res = bass_utils.run_bass_kernel_spmd(
        nc, in_maps, core_ids=list(range(NCORES)), trace=trace, **kw
    )
    y = np.concatenate(
        [np.asarray(r["yT"], dtype=np.float32).T for r in res.results], axis=0
    )
    return y, res


def kernel(**inputs) -> np.ndarray:
    y, _ = run(
        inputs["x"], inputs["W_ih"], inputs["W_hh"], inputs["b_ih"], inputs["b_hh"]
    )
    return y
